# revision 10
# baseline (speedup 1.0000x reference)
"""Trainium2 Bass kernel for nn_CholecMetric (segment_reduce).

Per-core (1 clip per NeuronCore, data-parallel over N=8):
  score[h,w] = (sum_p iog_max[p] * Gp[p,h,w]) / (sum_p Gp[p,h,w])
  where iog_max[p] = max_t |Gp_p & Gt_t| / |Gt_t|   (0 where undefined)

Layout: hw = k*512 + c, k in [0,128) on partitions, c in [0,512) free.
  gp sbuf [128, 33, 512] fp8 (slot 32 = ones) via chunked SWDGE cast
  DMAs. gt is pre-shuffled ON THE HOST into the canonical DoubleRow
  weights layout [k, j, i, t] (j = c-pair, i = pair element, t inner)
  and cast-DMA'd straight into sbuf - no on-chip repack, and its DMA
  rows are 128 per chunk instead of 2048.
  Intersections: 256 fp8 DoubleRow matmuls (c-pairs, effective K=256
  per instruction) accumulating psum[16,33]; col 32 = gt_area via the
  ones slot.
  cover: strided DVE tensor_reduce over p per c-range, overlapped with
  the stream; rcov = 1/max(cover, 0.5).
  epilogue num = sum_p w[p]*Gp[p]: DVE STT chain | ACT prescale -> DVE
  bf16 adds | ACT prescale -> GpSimd adds; all SBUF-resident.
"""

import numpy as np

import concourse.bass as bass
import concourse.bacc as bacc
import concourse.tile as tile
from concourse import mybir
from concourse.bass_utils import run_bass_kernel_spmd

N, P, T, H, W = 8, 32, 16, 256, 256
HW = H * W          # 65536
K, C = 128, 512     # hw = k*C + c
NCORES = 8

F32 = mybir.dt.float32
BF16 = mybir.dt.bfloat16
F8 = mybir.dt.float8e4
I32 = mybir.dt.int32
ALU = mybir.AluOpType
DR = mybir.MatmulPerfMode.DoubleRow

# c-chunk boundaries (SWDGE cast loads; gp paces the matmuls)
GT_CHUNKS = (0, 64, 192, 352, 512)
GP_CHUNKS = (0, 64, 192, 352, 480, 512)
# cover-reduce ranges on DVE, aligned to gp chunk availability; the tail
# piece is emitted AFTER the w-chain DVE ops so it stays off the
# critical path (DVE queues are in-order)
COV_RANGES = ((0, 192), (192, 352), (352, 480))
COV_TAIL = (480, 512)

# epilogue p-split: DVE STT chain, ACT prescale -> DVE bf16 adds,
# ACT prescale -> GpSimd bf16 adds
NP_DVE = 10
NP_ACT = 10
NP_GPS = P - NP_DVE - NP_ACT


def build():
    nc = bacc.Bacc("TRN2", target_bir_lowering=False, debug=False,
                   num_devices=1)
    gp_d = nc.dram_tensor("gp", [P, HW], I32, kind="ExternalInput")
    gt2_d = nc.dram_tensor("gt2", [K, T * HW // K], I32, kind="ExternalInput")
    id16_d = nc.dram_tensor("id16", [T, T], F32, kind="ExternalInput")
    id32_d = nc.dram_tensor("id32", [P, P], F32, kind="ExternalInput")
    out_d = nc.dram_tensor("score", [HW], F32, kind="ExternalOutput")

    gp_r = gp_d.rearrange("p (k c) -> k p c", c=C)        # [128, 32, 512]
    gt2_r = gt2_d.rearrange("k (j i t) -> k j i t", i=2, t=T)
    out_r = out_d.rearrange("(k c) -> k c", c=C)          # [128, 512]

    with tile.TileContext(nc) as tc:
        with (
            tc.tile_pool(name="main", bufs=1) as main,
            tc.tile_pool(name="psum", bufs=1, space="PSUM") as psum,
        ):
            gp_t = main.tile([K, P + 1, C], F8, tag="gp")
            gt2_t = main.tile([K, C // 2, 2, T], F8, tag="gt2")

            # chunked SWDGE cast loads (int32 -> fp8), gt before gp per
            # range so the matmul stream is paced by gp
            gt_it = iter(zip(GT_CHUNKS, GT_CHUNKS[1:]))
            gp_it = iter(zip(GP_CHUNKS, GP_CHUNKS[1:]))
            for (t0, t1), (p0c, p1c) in zip(gt_it, gp_it):
                nc.gpsimd.dma_start(out=gt2_t[:, t0 // 2:t1 // 2, :, :],
                                    in_=gt2_r[:, t0 // 2:t1 // 2, :, :])
                nc.gpsimd.dma_start(out=gp_t[:, 0:P, p0c:p1c],
                                    in_=gp_r[:, :, p0c:p1c])
            for p0c, p1c in gp_it:
                nc.gpsimd.dma_start(out=gp_t[:, 0:P, p0c:p1c],
                                    in_=gp_r[:, :, p0c:p1c])

            # constants: identities on the idle sync HWDGE queue
            id16 = main.tile([T, T], F32, tag="id16")
            id32 = main.tile([P, P], F32, tag="id32")
            ones128 = main.tile([1, K], F32, tag="ones128")
            nc.sync.dma_start(out=id16[:], in_=id16_d[:])
            nc.sync.dma_start(out=id32[:], in_=id32_d[:])
            nc.vector.memset(ones128[:], 1.0)
            nc.vector.memset(gp_t[:, P, :], 1.0)  # ones slot

            # intersections + gt_area: 256 DoubleRow matmuls (c-pairs)
            psum_i = psum.tile([T, P + 1], F32, tag="inters")
            gpv = gp_t[:, :, :]
            for j in range(C // 2):
                rhs = bass.AP(tensor=gpv.tensor, offset=gpv.offset + 2 * j,
                              ap=[gpv.ap[0], [1, 2], [C, P + 1]])
                nc.tensor.matmul(
                    psum_i[:], gt2_t[:, j, :, :], rhs,
                    start=(j == 0), stop=(j == C // 2 - 1), perf_mode=DR)

            # cover = sum_p Gp via strided DVE reduces per c-range;
            # rcov = 1/max(cover, 0.5) (exact for cover >= 1)
            covm = main.tile([K, C], F32, tag="covm")
            rcov = main.tile([K, C], F32, tag="rcov")
            for c0, c1 in COV_RANGES:
                v = gp_t[:, 0:P, c0:c1]
                sap = bass.AP(tensor=v.tensor, offset=v.offset,
                              ap=[v.ap[0], [1, c1 - c0], [C, P]])
                nc.vector.tensor_reduce(covm[:, c0:c1], sap,
                                        mybir.AxisListType.X, ALU.add)
                nc.vector.tensor_scalar_max(covm[:, c0:c1], covm[:, c0:c1],
                                            0.5)
                nc.vector.reciprocal(rcov[:, c0:c1], covm[:, c0:c1])

            # w-chain: iogs = inters/area, transpose, max_t, broadcast
            iog_all = main.tile([T, P + 1], F32, tag="iogall")
            nc.scalar.copy(iog_all[:], psum_i[:])
            rarea = main.tile([T, 1], F32, tag="rarea")
            nc.vector.tensor_scalar_max(rarea[:], iog_all[:, P:P + 1], 0.5)
            nc.vector.reciprocal(rarea[:], rarea[:])
            iogs = main.tile([T, P], F32, tag="iogs")
            nc.vector.tensor_scalar_mul(iogs[:], iog_all[:, 0:P],
                                        rarea[:, 0:1])
            psum_tr = psum.tile([P, T], F32, tag="tr")
            nc.tensor.transpose(psum_tr[:], iogs[:], id16[:])
            iomax = main.tile([P, 1], F32, tag="iomax")
            nc.vector.tensor_reduce(iomax[:], psum_tr[:],
                                    mybir.AxisListType.X, ALU.max)
            psum_wr = psum.tile([1, P], F32, tag="wr")
            nc.tensor.matmul(psum_wr[:], iomax[:], id32[:])
            w_row = main.tile([1, P], F32, tag="wrow")
            nc.scalar.copy(w_row[:], psum_wr[:])
            psum_wb = psum.tile([K, P], F32, tag="wb")
            nc.tensor.matmul(psum_wb[:], ones128[:], w_row[:])
            w_bc = main.tile([K, P], F32, tag="wbc")
            nc.vector.tensor_copy(w_bc[:], psum_wb[:])

            # cover tail (off the critical path: after the w-chain)
            c0, c1 = COV_TAIL
            v = gp_t[:, 0:P, c0:c1]
            sap = bass.AP(tensor=v.tensor, offset=v.offset,
                          ap=[v.ap[0], [1, c1 - c0], [C, P]])
            nc.vector.tensor_reduce(covm[:, c0:c1], sap,
                                    mybir.AxisListType.X, ALU.add)
            nc.vector.tensor_scalar_max(covm[:, c0:c1], covm[:, c0:c1], 0.5)
            nc.vector.reciprocal(rcov[:, c0:c1], covm[:, c0:c1])

            # num = sum_p w[p] * Gp[p], three chains over p
            acc_v = main.tile([K, C], F32, tag="accv")
            acc_a = main.tile([K, C], BF16, tag="acca")
            acc_g = main.tile([K, C], BF16, tag="accg")
            nc.vector.tensor_scalar_mul(acc_v[:], gp_t[:, 0, :],
                                        w_bc[:, 0:1])
            for p in range(1, NP_DVE):
                nc.vector.scalar_tensor_tensor(
                    acc_v[:], gp_t[:, p, :], w_bc[:, p:p + 1], acc_v[:],
                    ALU.mult, ALU.add)
            p0 = NP_DVE
            nc.scalar.mul(acc_a[:], gp_t[:, p0, :], w_bc[:, p0:p0 + 1])
            for p in range(p0 + 1, p0 + NP_ACT):
                at = main.tile([K, C], BF16, tag=f"at{p % 4}")
                nc.scalar.mul(at[:], gp_t[:, p, :], w_bc[:, p:p + 1])
                nc.vector.tensor_tensor(acc_a[:], acc_a[:], at[:], ALU.add)
            p0 = NP_DVE + NP_ACT
            nc.scalar.mul(acc_g[:], gp_t[:, p0, :], w_bc[:, p0:p0 + 1])
            for p in range(p0 + 1, P):
                gt_tmp = main.tile([K, C], BF16, tag=f"gat{p % 4}")
                nc.scalar.mul(gt_tmp[:], gp_t[:, p, :], w_bc[:, p:p + 1])
                nc.gpsimd.tensor_tensor(acc_g[:], acc_g[:], gt_tmp[:],
                                        ALU.add)

            # combine and divide
            nc.vector.tensor_tensor(acc_v[:], acc_v[:], acc_a[:], ALU.add)
            nc.vector.tensor_tensor(acc_v[:], acc_v[:], acc_g[:], ALU.add)
            score = main.tile([K, C], F32, tag="score")
            nc.vector.tensor_tensor(score[:], acc_v[:], rcov[:], ALU.mult)

            nc.sync.dma_start(out=out_r[:], in_=score[:])

    nc.compile()
    return nc


_NC_CACHE = None


def _get_nc():
    global _NC_CACHE
    if _NC_CACHE is None:
        _NC_CACHE = build()
    return _NC_CACHE


def kernel(groups_pred: np.ndarray, groups_true: np.ndarray, trace=False,
           **trace_kwargs) -> np.ndarray:
    nc = _get_nc()
    gp = np.ascontiguousarray(
        np.asarray(groups_pred, dtype=np.int32)).reshape(N, P, HW)
    gt = np.asarray(groups_true, dtype=np.int32).reshape(N, T, HW)
    # host pre-shuffle of gt into the DoubleRow weights layout
    # [k, j, i, t]: gt[n, t, k*C + 2j + i] -> gt2[n, k, j, i, t]
    gt2 = np.ascontiguousarray(
        gt.reshape(N, T, K, C // 2, 2).transpose(0, 2, 3, 4, 1)
    ).reshape(N, K, T * HW // K)
    id16 = np.eye(T, dtype=np.float32)
    id32 = np.eye(P, dtype=np.float32)
    in_maps = [{"gp": gp[n], "gt2": gt2[n], "id16": id16, "id32": id32}
               for n in range(N)]
    res = run_bass_kernel_spmd(nc, in_maps, list(range(NCORES)), trace=trace,
                               **trace_kwargs)
    out = np.stack([res.results[n]["score"].reshape(H, W) for n in range(N)])
    if trace:
        kernel.last_results = res
    return out.astype(np.float32)
